# revision 4
# baseline (speedup 1.0000x reference)
"""Trainium2 Bass kernel: NanoGPT causal self-attention, 8-core SPMD.

Sharding: mesh 2 (batch) x 4 (head groups of 4 heads). Each core computes
qkv projection for its 4 heads, causal attention, and a partial c_proj
(row-parallel TP); host sums the 4 partials per batch element.

Bias folding (validated vs reference in fp32 to 2e-6 rel):
 - k bias dropped (softmax shift-invariant per query row)
 - v bias + c_proj bias folded to host constant row: W_proj @ b_v + b_proj
 - q bias (scaled by 1/sqrt(D)) applied on-device per-partition.
"""
import sys

sys.path.insert(0, "/opt/trn_rl_repo")

import numpy as np
import ml_dtypes
from contextlib import ExitStack

import concourse.bass as bass
import concourse.mybir as mybir
import concourse.tile as tile
from concourse import bacc
from concourse.bass import ts, ds
from concourse.bass_utils import run_bass_kernel_spmd
from concourse.masks import make_identity, make_causal_mask

F32 = mybir.dt.float32
BF16 = mybir.dt.bfloat16
AF = mybir.ActivationFunctionType
AX = mybir.AxisListType

N_EMBD, N_HEAD, B, T = 1024, 16, 2, 2048
D = N_EMBD // N_HEAD      # 64
HPC = 4                   # heads per core
CL = HPC * D              # 256 local head dims
NC = 8                    # cores

bf16 = ml_dtypes.bfloat16


def emit_program(nc: bass.Bass):
    xT_d = nc.dram_tensor("xT", [N_EMBD, T], BF16, kind="ExternalInput").ap()
    wqk_d = nc.dram_tensor("wqk", [N_EMBD, 2 * CL], BF16, kind="ExternalInput").ap()
    wv_d = nc.dram_tensor("wv", [N_EMBD, CL], BF16, kind="ExternalInput").ap()
    wp_d = nc.dram_tensor("wp", [CL, N_EMBD], BF16, kind="ExternalInput").ap()
    bc_d = nc.dram_tensor("bc", [128, 4], F32, kind="ExternalInput").ap()
    out_d = nc.dram_tensor("out", [N_EMBD, T], F32, kind="ExternalOutput").ap()

    with tile.TileContext(nc) as tc, ExitStack() as ctx:
        persist = ctx.enter_context(tc.tile_pool(name="persist", bufs=1))
        work = ctx.enter_context(tc.tile_pool(name="work", bufs=2))
        pts = ctx.enter_context(tc.tile_pool(name="pts", bufs=4))
        small = ctx.enter_context(tc.tile_pool(name="small", bufs=4))

        xT = persist.tile([128, 8, T], BF16)          # x[b].T  (chan-major)
        wqk = persist.tile([128, 8, 2 * CL], BF16)
        wv = persist.tile([128, 8, CL], BF16)
        wp = persist.tile([128, 2, N_EMBD], BF16)
        bc = persist.tile([128, 4], F32)              # q-bias/8 (m=0,1), zeros
        qk = persist.tile([128, 4, T], BF16)          # qT (m 0,1) | kT (m 2,3)
        vsb = persist.tile([128, 16, CL], BF16)       # V natural [tok, vdim]
        ysb = persist.tile([128, 16, CL], BF16)       # y natural [tok, vdim]
        yT = persist.tile([128, 2, T], BF16)          # y transposed
        ident = persist.tile([128, 128], BF16)
        mask = persist.tile([128, 128], F32)          # strict-upper -1e9

        make_identity(nc, ident)
        make_causal_mask(nc, mask, mask_val=-1e9)
        nc.sync.dma_start(out=bc, in_=bc_d)
        for j in range(8):
            nc.sync.dma_start(out=xT[:, j, :], in_=xT_d[ts(j, 128), :])
            nc.sync.dma_start(out=wqk[:, j, :], in_=wqk_d[ts(j, 128), :])
            nc.sync.dma_start(out=wv[:, j, :], in_=wv_d[ts(j, 128), :])
        for j in range(2):
            nc.sync.dma_start(out=wp[:, j, :], in_=wp_d[ts(j, 128), :])

        # ---- phase 1: qkT = (Wqk^T x^T) and V = x Wv^T -------------------
        with tc.tile_pool(name="ps1", bufs=3, space="PSUM") as ps1:
            for m in range(4):
                for t in range(4):
                    qk_ps = ps1.tile([128, 512], F32, tag="qk_ps")
                    for j in range(8):
                        nc.tensor.matmul(
                            qk_ps, lhsT=wqk[:, j, ts(m, 128)],
                            rhs=xT[:, j, ts(t, 512)],
                            start=(j == 0), stop=(j == 7))
                    if m < 2:  # q rows: scale 1/8, add bias/8
                        nc.scalar.activation(
                            out=qk[:, m, ts(t, 512)], in_=qk_ps,
                            func=AF.Identity, bias=bc[:, m:m + 1], scale=0.125)
                    else:      # k rows: plain copy
                        nc.scalar.copy(out=qk[:, m, ts(t, 512)], in_=qk_ps)
            for t in range(16):
                v_ps = ps1.tile([128, CL], F32, tag="v_ps")
                for j in range(8):
                    nc.tensor.matmul(
                        v_ps, lhsT=xT[:, j, ts(t, 128)], rhs=wv[:, j, :],
                        start=(j == 0), stop=(j == 7))
                nc.scalar.copy(out=vsb[:, t, :], in_=v_ps)

        # ---- phase 2: causal attention per head --------------------------
        with tc.tile_pool(name="ps2", bufs=1, space="PSUM") as ps2:
            for h in range(HPC):
                po = 64 * (h % 2)
                mq = h // 2
                mk = 2 + h // 2
                for qi in range(16):
                    kl = 128 * (qi + 1)
                    nk = (kl + 511) // 512
                    p_sb = work.tile([128, T], BF16, tag="p_sb")
                    lpart = small.tile([128, 4], F32, tag="lpart")
                    s_list = []
                    for ci in range(nk):
                        n_c = min(512, kl - 512 * ci)
                        s_ps = ps2.tile([128, 512], F32, tag="s_ps", bufs=4)
                        nc.tensor.matmul(
                            s_ps[:, :n_c],
                            lhsT=qk[po:po + 64, mq, ts(qi, 128)],
                            rhs=qk[po:po + 64, mk, ds(512 * ci, n_c)],
                            start=True, stop=True)
                        s_list.append((s_ps, n_c))
                    # causal mask: last 128 keys of the row = diagonal block
                    s_ps_l, n_c_l = s_list[-1]
                    nc.vector.tensor_add(
                        out=s_ps_l[:, n_c_l - 128:n_c_l],
                        in0=s_ps_l[:, n_c_l - 128:n_c_l], in1=mask)
                    for ci, (s_ps, n_c) in enumerate(s_list):
                        nc.scalar.activation(
                            out=p_sb[:, ds(512 * ci, n_c)], in_=s_ps[:, :n_c],
                            func=AF.Exp, accum_out=lpart[:, ci:ci + 1])
                    l = small.tile([128, 1], F32, tag="l")
                    nc.vector.reduce_sum(out=l, in_=lpart[:, :nk], axis=AX.X)
                    linv = small.tile([128, 1], F32, tag="linv")
                    nc.vector.reciprocal(out=linv, in_=l)
                    y_ps = ps2.tile([128, D], F32, tag="y_ps", bufs=2)
                    for kc in range(qi + 1):
                        pT_ps = ps2.tile([128, 128], BF16, tag="pT_ps", bufs=2)
                        nc.tensor.transpose(
                            out=pT_ps, in_=p_sb[:, ts(kc, 128)], identity=ident)
                        pT_sb = pts.tile([128, 128], BF16, tag="pT_sb")
                        nc.vector.tensor_copy(out=pT_sb, in_=pT_ps)
                        nc.tensor.matmul(
                            y_ps, lhsT=pT_sb, rhs=vsb[:, kc, ds(D * h, D)],
                            start=(kc == 0), stop=(kc == qi))
                    nc.scalar.mul(out=ysb[:, qi, ds(D * h, D)], in_=y_ps,
                                  mul=linv)
            # y -> yT transposes for the projection
            for t in range(16):
                for kd in range(2):
                    tr_ps = ps2.tile([128, 128], BF16, tag="pT_ps", bufs=2)
                    nc.tensor.transpose(
                        out=tr_ps, in_=ysb[:, t, ts(kd, 128)], identity=ident)
                    nc.vector.tensor_copy(out=yT[:, kd, ts(t, 128)], in_=tr_ps)

        # ---- phase 3: out^T = WpL^T y^T ---------------------------------
        with tc.tile_pool(name="ps3", bufs=4, space="PSUM") as ps3:
            for m in range(8):
                for t in range(4):
                    o_ps = ps3.tile([128, 512], F32, tag="o_ps")
                    for kd in range(2):
                        nc.tensor.matmul(
                            o_ps, lhsT=wp[:, kd, ts(m, 128)],
                            rhs=yT[:, kd, ts(t, 512)],
                            start=(kd == 0), stop=(kd == 1))
                    o_sb = work.tile([128, 512], F32, tag="o_sb", bufs=3)
                    nc.scalar.copy(out=o_sb, in_=o_ps)
                    nc.sync.dma_start(out=out_d[ts(m, 128), ts(t, 512)],
                                      in_=o_sb)
    return nc


_program = None


def get_program() -> bass.Bass:
    global _program
    if _program is None:
        nc = bacc.Bacc("TRN2", target_bir_lowering=False, debug=False,
                       num_devices=NC)
        emit_program(nc)
        nc.compile()
        _program = nc
    return _program


def make_in_maps(x, W_attn, b_attn, W_proj, b_proj):
    x = np.asarray(x, np.float32)
    W_attn = np.asarray(W_attn, np.float32)
    b_attn = np.asarray(b_attn, np.float32)
    W_proj = np.asarray(W_proj, np.float32)

    xTs = [np.ascontiguousarray(x[b].T).astype(bf16) for b in range(B)]
    per_g = []
    for g in range(4):
        qrows = slice(CL * g, CL * (g + 1))
        krows = slice(N_EMBD + CL * g, N_EMBD + CL * (g + 1))
        vrows = slice(2 * N_EMBD + CL * g, 2 * N_EMBD + CL * (g + 1))
        wqkT = np.concatenate([W_attn[qrows], W_attn[krows]], 0).T
        wvT = W_attn[vrows].T
        wpLT = W_proj[:, qrows].T
        bc = np.zeros((128, 4), np.float32)
        bc[:, 0:2] = (b_attn[qrows] * 0.125).reshape(2, 128).T
        per_g.append({
            "wqk": np.ascontiguousarray(wqkT).astype(bf16),
            "wv": np.ascontiguousarray(wvT).astype(bf16),
            "wp": np.ascontiguousarray(wpLT).astype(bf16),
            "bc": bc,
        })
    in_maps = []
    for c in range(NC):
        b, g = divmod(c, 4)
        m = dict(per_g[g])
        m["xT"] = xTs[b]
        in_maps.append(m)
    return in_maps


def kernel(x, W_attn, b_attn, W_proj, b_proj):
    in_maps = make_in_maps(x, W_attn, b_attn, W_proj, b_proj)
    nc = get_program()
    res = run_bass_kernel_spmd(nc, in_maps, core_ids=list(range(NC)))
    W_proj = np.asarray(W_proj, np.float32)
    b_attn = np.asarray(b_attn, np.float32)
    b_proj = np.asarray(b_proj, np.float32)
    const_row = W_proj @ b_attn[2 * N_EMBD:] + b_proj
    out = np.zeros((B, T, N_EMBD), np.float32)
    for c in range(NC):
        b = c // 4
        out[b] += res.results[c]["out"].T
    out += const_row[None, None, :]
    return out


# revision 7
# speedup vs baseline: 1.5621x; 1.5621x over previous
"""Trainium2 Bass kernel: NanoGPT causal self-attention, 8-core SPMD.

Sharding: mesh 2 (batch) x 4 (head groups of 4 heads). Each core computes
qkv projection for its 4 heads, causal attention, and a partial c_proj
(row-parallel TP); host sums the 4 partials per batch element.

Bias folding (validated vs reference in fp32 to 2e-6 rel):
 - k bias dropped (softmax shift-invariant per query row)
 - v bias + c_proj bias folded to host constant row: W_proj @ b_v + b_proj
 - q bias (scaled by 1/sqrt(D)) applied on-device per-partition.

Attention is computed in keys-major orientation: s^T[key, q] chunks come
straight out of the PE (lhsT = k^T block, rhs = q^T), exp goes PSUM->SBUF
on the scalar engine producing p^T, which is directly the stationary
operand for the p@V matmul — no PE transposes of p needed. The softmax
denominator comes for free from a ones-column appended to V: column 64 of
the [128, 65] PV output accumulates sum(p) per query row, per-partition,
exactly where the 1/l scale needs it.
"""
import sys

sys.path.insert(0, "/opt/trn_rl_repo")

import numpy as np
import ml_dtypes
from contextlib import ExitStack

import concourse.bass as bass
import concourse.mybir as mybir
import concourse.tile as tile
from concourse import bacc
from concourse.bass import ts, ds
from concourse.bass_utils import run_bass_kernel_spmd
from concourse.masks import make_identity

F32 = mybir.dt.float32
BF16 = mybir.dt.bfloat16
AF = mybir.ActivationFunctionType

N_EMBD, N_HEAD, B, T = 1024, 16, 2, 2048
D = N_EMBD // N_HEAD      # 64
HPC = 4                   # heads per core
CL = HPC * D              # 256 local head dims
NC = 8                    # cores
NEG = -1.0e9

bf16 = ml_dtypes.bfloat16


def emit_program(nc: bass.Bass):
    xT_d = nc.dram_tensor("xT", [N_EMBD, T], BF16, kind="ExternalInput").ap()
    wqk_d = nc.dram_tensor("wqk", [N_EMBD, 2 * CL], BF16, kind="ExternalInput").ap()
    wv_d = nc.dram_tensor("wv", [N_EMBD, CL], BF16, kind="ExternalInput").ap()
    wp_d = nc.dram_tensor("wp", [CL, N_EMBD], BF16, kind="ExternalInput").ap()
    bc_d = nc.dram_tensor("bc", [128, 4], F32, kind="ExternalInput").ap()
    out_d = nc.dram_tensor("out", [N_EMBD, T], F32, kind="ExternalOutput").ap()

    with tile.TileContext(nc) as tc, ExitStack() as ctx:
        persist = ctx.enter_context(tc.tile_pool(name="persist", bufs=1))
        work = ctx.enter_context(tc.tile_pool(name="work", bufs=2))
        ppool = ctx.enter_context(tc.tile_pool(name="ppool", bufs=2))
        small = ctx.enter_context(tc.tile_pool(name="small", bufs=4))
        psum = ctx.enter_context(tc.tile_pool(name="psum", bufs=4,
                                              space="PSUM"))

        xT = persist.tile([128, 8, T], BF16)          # x[b].T  (chan-major)
        wqk = persist.tile([128, 8, 2 * CL], BF16)
        wv = persist.tile([128, 8, CL], BF16)
        wp = persist.tile([128, 2, N_EMBD], BF16)
        bc = persist.tile([128, 4], F32)              # q-bias/8 (m=0,1), zeros
        qk = persist.tile([128, 4, T], BF16)          # qT (m 0,1) | kT (m 2,3)
        vsb = persist.tile([128, 16, HPC, D + 2], BF16)  # V | ones | zero pad
        ysb = persist.tile([128, 16, CL], BF16)       # y natural [tok, vdim]
        yT = persist.tile([128, 2, T], BF16)          # y transposed
        ident = persist.tile([128, 128], BF16)
        maskT = persist.tile([128, 128], F32)         # strict-lower -1e9

        make_identity(nc, ident)
        # s^T orientation causal mask: keep where query(col) >= key(row)
        nc.gpsimd.memset(maskT, 0.0)
        nc.gpsimd.affine_select(
            out=maskT, in_=maskT, compare_op=mybir.AluOpType.is_ge,
            fill=NEG, base=0, pattern=[[1, 128]], channel_multiplier=-1)
        nc.sync.dma_start(out=bc, in_=bc_d)
        for j in range(8):
            nc.sync.dma_start(out=xT[:, j, :], in_=xT_d[ts(j, 128), :])
            nc.sync.dma_start(out=wqk[:, j, :], in_=wqk_d[ts(j, 128), :])
            nc.sync.dma_start(out=wv[:, j, :], in_=wv_d[ts(j, 128), :])
        for j in range(2):
            nc.sync.dma_start(out=wp[:, j, :], in_=wp_d[ts(j, 128), :])
        nc.vector.memset(vsb[:, :, :, D:D + 1], 1.0)
        nc.vector.memset(vsb[:, :, :, D + 1:D + 2], 0.0)

        # ---- phase 1: qkT = (Wqk^T x^T) and V = x Wv^T -------------------
        for m in range(4):
            qk_ps = [psum.tile([128, 512], F32, tag="mm512", name=f"qk_ps{m}_{t}")
                     for t in range(4)]
            for j in range(8):
                for t in range(4):
                    nc.tensor.matmul(
                        qk_ps[t], lhsT=wqk[:, j, ts(m, 128)],
                        rhs=xT[:, j, ts(t, 512)],
                        start=(j == 0), stop=(j == 7))
            for t in range(4):
                if m < 2:  # q rows: scale 1/8, add bias/8 (ACT, per-partition)
                    nc.scalar.activation(
                        out=qk[:, m, ts(t, 512)], in_=qk_ps[t],
                        func=AF.Identity, bias=bc[:, m:m + 1], scale=0.125)
                else:      # k rows: plain copy on DVE
                    nc.vector.tensor_copy(out=qk[:, m, ts(t, 512)],
                                          in_=qk_ps[t])
        for t in range(16):
            v_ps = psum.tile([128, CL], F32, tag="v_ps", bufs=2)
            for j in range(8):
                nc.tensor.matmul(
                    v_ps, lhsT=xT[:, j, ts(t, 128)], rhs=wv[:, j, :],
                    start=(j == 0), stop=(j == 7))
            nc.vector.tensor_copy(
                out=vsb[:, t, :, 0:D],
                in_=v_ps.rearrange("p (h d) -> p h d", h=HPC))

        # ---- phase 2: causal attention, keys-major ----------------------
        for h in range(HPC):
            po = 64 * (h % 2)
            mq = h // 2
            mk = 2 + h // 2
            for half in range(2):
                qbase = 1024 * half
                kcn = 8 * (half + 1)   # key chunks in this half
                pT = ppool.tile([128, 16, 1024], BF16, tag="pT")
                for kc in range(kcn):
                    q_lo = max(128 * kc, qbase)
                    for blk in range(2):
                        b0 = qbase + 512 * blk
                        off = max(q_lo - b0, 0)
                        if off >= 512:
                            continue
                        w = 512 - off
                        sT_ps = psum.tile([128, 512], F32, tag="mm512",
                                          name=f"sT{h}_{half}_{kc}_{blk}")
                        nc.tensor.matmul(
                            sT_ps[:, off:512],
                            lhsT=qk[po:po + 64, mk, ds(128 * kc, 128)],
                            rhs=qk[po:po + 64, mq, ds(b0 + off, w)],
                            start=True, stop=True)
                        # diagonal block: mask keys > query
                        if b0 <= 128 * kc < b0 + 512:
                            doff = 128 * kc - b0
                            nc.vector.tensor_add(
                                out=sT_ps[:, doff:doff + 128],
                                in0=sT_ps[:, doff:doff + 128], in1=maskT)
                        nc.scalar.activation(
                            out=pT[:, kc, ds(b0 - qbase + off, w)],
                            in_=sT_ps[:, off:512], func=AF.Exp)
                for ql in range(8):
                    qi = 8 * half + ql
                    y_ps = psum.tile([128, D + 2], F32, tag="y_ps", bufs=2)
                    for kc in range(qi + 1):
                        nc.tensor.matmul(
                            y_ps, lhsT=pT[:, kc, ts(ql, 128)],
                            rhs=vsb[:, kc, h, :],
                            start=(kc == 0), stop=(kc == qi))
                    linv = small.tile([128, 1], F32, tag="linv")
                    nc.vector.reciprocal(out=linv, in_=y_ps[:, D:D + 1])
                    nc.vector.tensor_scalar_mul(
                        out=ysb[:, qi, ds(D * h, D)], in0=y_ps[:, 0:D],
                        scalar1=linv)
            # y -> yT transposes as soon as both heads of a kd-half are done
            if h == 1 or h == 3:
                kd = h // 2
                for t in range(16):
                    tr_ps = psum.tile([128, 128], BF16, tag="y_ps", bufs=2,
                                      name=f"tr_ps{h}_{t}")
                    nc.tensor.transpose(
                        out=tr_ps, in_=ysb[:, t, ts(kd, 128)], identity=ident)
                    nc.vector.tensor_copy(out=yT[:, kd, ts(t, 128)],
                                          in_=tr_ps)

        # ---- phase 3: out^T = WpL^T y^T ---------------------------------
        for m in range(8):
            o_ps = [psum.tile([128, 512], F32, tag="mm512", name=f"o_ps{m}_{t}")
                    for t in range(4)]
            for kd in range(2):
                for t in range(4):
                    nc.tensor.matmul(
                        o_ps[t], lhsT=wp[:, kd, ts(m, 128)],
                        rhs=yT[:, kd, ts(t, 512)],
                        start=(kd == 0), stop=(kd == 1))
            for t in range(4):
                o_sb = work.tile([128, 512], F32, tag="o_sb", bufs=3)
                nc.vector.tensor_copy(out=o_sb, in_=o_ps[t])
                nc.sync.dma_start(out=out_d[ts(m, 128), ts(t, 512)],
                                  in_=o_sb)
    return nc


_program = None


def get_program() -> bass.Bass:
    global _program
    if _program is None:
        nc = bacc.Bacc("TRN2", target_bir_lowering=False, debug=False,
                       num_devices=NC)
        emit_program(nc)
        nc.compile()
        _program = nc
    return _program


def make_in_maps(x, W_attn, b_attn, W_proj, b_proj):
    x = np.asarray(x, np.float32)
    W_attn = np.asarray(W_attn, np.float32)
    b_attn = np.asarray(b_attn, np.float32)
    W_proj = np.asarray(W_proj, np.float32)

    xTs = [np.ascontiguousarray(x[b].T).astype(bf16) for b in range(B)]
    per_g = []
    for g in range(4):
        qrows = slice(CL * g, CL * (g + 1))
        krows = slice(N_EMBD + CL * g, N_EMBD + CL * (g + 1))
        vrows = slice(2 * N_EMBD + CL * g, 2 * N_EMBD + CL * (g + 1))
        wqkT = np.concatenate([W_attn[qrows], W_attn[krows]], 0).T
        wvT = W_attn[vrows].T
        wpLT = W_proj[:, qrows].T
        bc = np.zeros((128, 4), np.float32)
        bc[:, 0:2] = (b_attn[qrows] * 0.125).reshape(2, 128).T
        per_g.append({
            "wqk": np.ascontiguousarray(wqkT).astype(bf16),
            "wv": np.ascontiguousarray(wvT).astype(bf16),
            "wp": np.ascontiguousarray(wpLT).astype(bf16),
            "bc": bc,
        })
    in_maps = []
    for c in range(NC):
        b, g = divmod(c, 4)
        m = dict(per_g[g])
        m["xT"] = xTs[b]
        in_maps.append(m)
    return in_maps


def kernel(x, W_attn, b_attn, W_proj, b_proj):
    in_maps = make_in_maps(x, W_attn, b_attn, W_proj, b_proj)
    nc = get_program()
    res = run_bass_kernel_spmd(nc, in_maps, core_ids=list(range(NC)))
    W_proj = np.asarray(W_proj, np.float32)
    b_attn = np.asarray(b_attn, np.float32)
    b_proj = np.asarray(b_proj, np.float32)
    const_row = W_proj @ b_attn[2 * N_EMBD:] + b_proj
    out = np.zeros((B, T, N_EMBD), np.float32)
    for c in range(NC):
        b = c // 4
        out[b] += res.results[c]["out"].T
    out += const_row[None, None, :]
    return out
